# revision 19
# baseline (speedup 1.0000x reference)
"""Trainium2 Bass kernel for the inverse deep-hough-transform gather-reduce.

out[n, c, y, x] = sum_k acc[n, c, k, rho_idx[k, y, x]]  (masked by validity)

Design (v2)
-----------
- The rho index table is a pure function of static shapes; precomputed on the
  host.  Validity is folded in by pointing invalid entries at a zeroed spare
  slot (block R) of the data table.
- Gather primitive: GPSIMD IndirectCopy.  Measured cost is per 3-index
  read-request (~102-cycle serial RD_CMD), so each index fetches a contiguous
  block of F=32 nc-values -> 3.6us per 32-index IC (1024 elems/partition, the
  ISA cap per IC).
- Sharding: every core holds ALL 512 (n,c) rows; angles are sharded as
  k === core (mod 8).  Host sums the 8 per-core partials (the unshard step).
- Layout: 128 partitions = 8 groups x 16.  At step s, group g processes angle
  slot s*8+g (3 steps x 8 groups = 24 slots >= 23 angles/core; extra slots
  gather zeros).  Partition p = g*16+u plus block lane f in [0,32) covers
  nc = f*16+u.
- Per yx-chunk (64 positions): GPSIMD gathers [128, 2048] per step, DVE
  accumulates over steps, PE reduces the 8 groups with an exact 0/1 fp32
  selection matmul into PSUM, ACT copies PSUM->SBUF, sync DMA dumps to HBM.
- Raw Bass blocks with explicit semaphores (this walrus build allows at most
  one attached sync-wait per engine instruction; standalone EVSEM waits are
  used instead).
"""

from contextlib import ExitStack

import numpy as np

import concourse.bass as bass
from concourse import mybir
from concourse.bass_utils import run_bass_kernel_spmd

# Problem constants (hardcoded per the harness contract).
N, C, A, R = 4, 128, 180, 184
H = W = 128
YX = H * W  # 16384
NCORES = 8

GROUPS = 8  # 16-partition groups
U = 16  # partitions per group
F = 32  # nc values packed per rho block (IC inner size)
NCPC = F * U  # 512 nc rows held per core (all of them)
STEPS = 3  # angle slots per core = STEPS*GROUPS = 24 >= ceil(180/8)
ASLOT = STEPS * GROUPS  # 24
RPAD = R + 1  # 185 blocks; block R is all-zeros (invalid sink)
CHUNK = 64  # yx positions per chunk
NCH = YX // CHUNK  # 256 chunks
DW = RPAD * F  # data words per step per partition (5920)
CW = CHUNK // U  # idx columns per (step, chunk) per partition (4)
ICSUB = 1024 // F  # indices per IC (32): ISA caps IC dst at 1024 elems
NSUB = CHUNK // ICSUB  # sub-ICs per (chunk, step) (2)

_cache = {}


def _core_angles(core):
    """Angle slots for this core: slot t (0..23) -> global k or None."""
    ks = [k for k in range(A) if k % NCORES == core]
    return [ks[t] if t < len(ks) else None for t in range(ASLOT)]


def _rho_block_table():
    """[A, YX] int32 block indices into the padded rho axis (R = invalid)."""
    if "blk" in _cache:
        return _cache["blk"]
    k = np.arange(A)
    theta = k * (np.pi / A)
    cos_t = np.cos(theta)
    sin_t = np.sin(theta)
    y, x = np.meshgrid(np.arange(H), np.arange(W), indexing="ij")
    xc = (x - W // 2).astype(np.float64)
    yc = (y - H // 2).astype(np.float64)
    r = np.round(cos_t[:, None, None] * xc[None] + sin_t[:, None, None] * yc[None])
    r = r.astype(np.int64) + R // 2  # [A, H, W]
    valid = (r >= 0) & (r < R)
    blk = np.where(valid, np.clip(r, 0, R - 1), R).astype(np.int32)
    _cache["blk"] = blk.reshape(A, YX)
    return _cache["blk"]


def _idx_table(core):
    """uint16 idx stream for one core, SBUF layout [128, STEPS*NCH*CW].

    IndirectCopy unwraps a 16-partition group's idx tile as
    (col*16 + p_in_group); values are flat element offsets = block*F.
    Column layout: col = s*(NCH*CW) + q*CW + w.
    """
    key = ("idx", core)
    if key in _cache:
        return _cache[key]
    blk = _rho_block_table()
    angles = _core_angles(core)
    out = np.full((128, STEPS * NCH * CW), R * F, np.uint16)
    for s in range(STEPS):
        for g in range(GROUPS):
            k = angles[s * GROUPS + g]
            if k is None:
                continue
            flat = (blk[k] * F).astype(np.uint16)  # [YX]
            v = flat.reshape(NCH, CW, U)  # [q, w, p_in_group]
            v = v.transpose(2, 0, 1).reshape(U, NCH * CW)
            cols = slice(s * NCH * CW, (s + 1) * NCH * CW)
            out[g * U : (g + 1) * U, cols] = v
    _cache[key] = out
    return out


def _data_for_core(acc_flat, core):
    """acc_flat [512, A, R] f32 -> [128, STEPS*DW] f32 padded gather table.

    data[g*16+u, s*DW + rb*F + f] = acc_flat[f*16+u, k(s*8+g), rb]
    """
    angles = _core_angles(core)
    ac = np.zeros((NCPC, ASLOT, RPAD), np.float32)
    for t, k in enumerate(angles):
        if k is not None:
            ac[:, t, :R] = acc_flat[:, k, :]
    t = ac.reshape(F, U, ASLOT, RPAD)  # [f, u, t, r]
    t = t.transpose(2, 1, 3, 0)  # [t, u, r, f]
    t = t.reshape(STEPS, GROUPS, U, RPAD, F)  # [s, g, u, r, f]
    t = t.reshape(STEPS, 128, DW)
    return np.ascontiguousarray(t.transpose(1, 0, 2).reshape(128, STEPS * DW))


def _sel_matrix():
    """[128, 16] f32 selection: S[p, m] = 1 if p % 16 == m (group reduce)."""
    s = np.zeros((128, U), np.float32)
    s[np.arange(128), np.arange(128) % U] = 1.0
    return s


def _build_nc():
    if "nc" in _cache:
        return _cache["nc"]
    nc = bass.Bass("TRN2", debug=False, target_bir_lowering=False, num_devices=NCORES)
    data_d = nc.dram_tensor(
        "data", [128, STEPS * DW], mybir.dt.float32, kind="ExternalInput"
    ).ap()
    idx_d = nc.dram_tensor(
        "idx", [128, STEPS * NCH * CW], mybir.dt.uint16, kind="ExternalInput"
    ).ap()
    sel_d = nc.dram_tensor(
        "sel", [128, U], mybir.dt.float32, kind="ExternalInput"
    ).ap()
    raw_d = nc.dram_tensor(
        "raw", [NCH, U, CHUNK * F], mybir.dt.float32, kind="ExternalOutput"
    ).ap()

    GW = CHUNK * F  # 2048 gather/accum words per partition per (chunk, step)
    NMM = GW // 512  # matmuls per chunk (PSUM bank = 512 fp32)

    ctx = ExitStack()
    _cache["ctx"] = ctx
    data_sb = ctx.enter_context(nc.sbuf_tensor("data_sb", [128, STEPS * DW], mybir.dt.float32))
    idx_sb = ctx.enter_context(
        nc.sbuf_tensor("idx_sb", [128, STEPS * NCH * CW], mybir.dt.uint16)
    )
    sel_sb = ctx.enter_context(nc.sbuf_tensor("sel_sb", [128, U], mybir.dt.float32))
    gbuf = [
        ctx.enter_context(nc.sbuf_tensor(f"gbuf{i}", [128, GW], mybir.dt.float32))
        for i in range(2)
    ]
    abuf = [
        ctx.enter_context(nc.sbuf_tensor(f"abuf{i}", [128, GW], mybir.dt.float32))
        for i in range(2)
    ]
    obuf = [
        ctx.enter_context(nc.sbuf_tensor(f"obuf{i}", [U, GW], mybir.dt.float32))
        for i in range(2)
    ]
    psum = [
        ctx.enter_context(nc.psum_tensor(f"ps{i}", [U, GW], mybir.dt.float32))
        for i in range(2)
    ]
    ld_sem = ctx.enter_context(nc.semaphore("ld_sem"))
    ic_sem = ctx.enter_context(nc.semaphore("ic_sem"))
    add_sem = ctx.enter_context(nc.semaphore("add_sem"))
    mm_sem = ctx.enter_context(nc.semaphore("mm_sem"))
    cp_sem = ctx.enter_context(nc.semaphore("cp_sem"))
    dump_sem = ctx.enter_context(nc.semaphore("dump_sem"))
    block = ctx.enter_context(nc.Block())

    @block.gpsimd
    def _(gpsimd):
        gpsimd.dma_start(data_sb[:], data_d[:]).then_inc(ld_sem, 16)
        gpsimd.dma_start(idx_sb[:], idx_d[:]).then_inc(ld_sem, 16)
        gpsimd.dma_start(sel_sb[:], sel_d[:]).then_inc(ld_sem, 16)
        gpsimd.wait_ge(ld_sem, 48)
        j = 0  # (chunk, step) counter
        for q in range(NCH):
            for s in range(STEPS):
                if j >= 2:
                    gpsimd.wait_ge(add_sem, j - 1)
                dslice = data_sb[:, s * DW : (s + 1) * DW]
                ibase = s * NCH * CW + q * CW
                for sub in range(NSUB):
                    cw2 = CW // NSUB  # idx cols per sub-IC
                    isl = idx_sb[
                        :, ibase + sub * cw2 : ibase + (sub + 1) * cw2
                    ]
                    gpsimd.indirect_copy(
                        out=gbuf[j % 2][
                            :, sub * (GW // NSUB) : (sub + 1) * (GW // NSUB)
                        ].rearrange("p (i f) -> p i f", f=F),
                        data=dslice.rearrange("p (r f) -> p r f", f=F),
                        idxs=isl,
                        i_know_ap_gather_is_preferred=True,
                    ).then_inc(ic_sem, 1)
                j += 1

    @block.vector
    def _(vector):
        j = 0
        for q in range(NCH):
            acc = abuf[q % 2]
            for s in range(STEPS):
                vector.wait_ge(ic_sem, (j + 1) * NSUB)
                g_ = gbuf[j % 2]
                if s == 0:
                    # accum slot reused: PE must be done reading chunk q-2
                    if q >= 2:
                        vector.wait_ge(mm_sem, (q - 1) * NMM)
                    vector.tensor_copy(acc[:], g_[:]).then_inc(add_sem, 1)
                else:
                    vector.tensor_add(acc[:], acc[:], g_[:]).then_inc(add_sem, 1)
                j += 1

    @block.tensor
    def _(tensor):
        for q in range(NCH):
            tensor.wait_ge(add_sem, (q + 1) * STEPS)
            if q >= 2:
                tensor.wait_ge(cp_sem, q - 1)  # psum slot reused
            for m in range(NMM):
                tensor.matmul(
                    out=psum[q % 2][:, m * 512 : (m + 1) * 512],
                    lhsT=sel_sb[:],
                    rhs=abuf[q % 2][:, m * 512 : (m + 1) * 512],
                    start=True,
                    stop=True,
                ).then_inc(mm_sem, 1)

    @block.scalar
    def _(scalar):
        for q in range(NCH):
            scalar.wait_ge(mm_sem, (q + 1) * NMM)
            if q >= 2:
                scalar.wait_ge(dump_sem, (q - 1) * 16)  # obuf slot reused
            scalar.copy(obuf[q % 2][:], psum[q % 2][:]).then_inc(cp_sem, 1)

    @block.sync
    def _(sync):
        for q in range(NCH):
            sync.wait_ge(cp_sem, q + 1)
            sync.dma_start(raw_d[q], obuf[q % 2][:]).then_inc(dump_sem, 16)

    _cache["nc"] = nc
    return nc


def _install_ntff_hook():
    """Provide the antenv.axon_hooks shim the image lacks, wiring the
    ctypes NTFF profiler from trn_agent_boot."""
    import sys
    import types

    if "antenv.axon_hooks" in sys.modules:
        return
    import antenv
    from trn_agent_boot.trn_boot import _ntff_profile_via_ctypes

    mod = types.ModuleType("antenv.axon_hooks")
    hook = _ntff_profile_via_ctypes("/opt/axon/libaxon_pjrt.so")
    mod.get_axon_ntff_profile_hook = lambda: hook
    mod.set_axon_ntff_profile_hook = lambda h: None
    sys.modules["antenv.axon_hooks"] = mod
    antenv.axon_hooks = mod


def hw_exec_time_ns(trace_cores=None):
    """Re-run the last kernel() invocation with tracing; return max core ns."""
    _install_ntff_hook()
    nc = _cache["nc"]
    res = run_bass_kernel_spmd(
        nc,
        _cache["in_maps"],
        core_ids=list(range(NCORES)),
        trace=True,
        trace_cores=trace_cores,
    )
    _cache["trace"] = res
    return res.exec_time_ns


def kernel(accumulator, out_H=128, out_W=128, numangle=180, numrho=184):
    accumulator = np.asarray(accumulator, np.float32)
    assert accumulator.shape == (N, C, A, R), accumulator.shape
    assert int(out_H) == H and int(out_W) == W
    assert int(numangle) == A and int(numrho) == R

    nc = _build_nc()
    acc_flat = np.ascontiguousarray(accumulator.reshape(N * C, A, R))
    sel = _sel_matrix()
    in_maps = [
        {
            "data": _data_for_core(acc_flat, core),
            "idx": _idx_table(core),
            "sel": sel,
        }
        for core in range(NCORES)
    ]
    _cache["in_maps"] = in_maps
    res = run_bass_kernel_spmd(nc, in_maps, core_ids=list(range(NCORES)))

    # Unshard: sum the 8 per-core partials.
    # raw[q, u, i*F + f] = partial for nc = f*16+u, yx = q*CHUNK+i
    total = np.zeros((NCPC, YX), np.float64)
    for core in range(NCORES):
        raw = res.results[core]["raw"]  # [NCH, U, CHUNK*F]
        oc = raw.reshape(NCH, U, CHUNK, F).transpose(3, 1, 0, 2).reshape(NCPC, YX)
        total += oc
    return total.astype(np.float32).reshape(N, C, H, W)


# revision 21
# speedup vs baseline: 1.0350x; 1.0350x over previous
"""Trainium2 Bass kernel for the inverse deep-hough-transform gather-reduce.

out[n, c, y, x] = sum_k acc[n, c, k, rho_idx[k, y, x]]  (masked by validity)

Design (v2)
-----------
- The rho index table is a pure function of static shapes; precomputed on the
  host.  Validity is folded in by pointing invalid entries at a zeroed spare
  slot (block R) of the data table.
- Gather primitive: GPSIMD IndirectCopy.  Measured cost is per 3-index
  read-request (~102-cycle serial RD_CMD), so each index fetches a contiguous
  block of F=32 nc-values -> 3.6us per 32-index IC (1024 elems/partition, the
  ISA cap per IC).
- Sharding: every core holds ALL 512 (n,c) rows; angles are sharded as
  k === core (mod 8).  Host sums the 8 per-core partials (the unshard step).
- Layout: 128 partitions = 8 groups x 16.  At step s, group g processes angle
  slot s*8+g (3 steps x 8 groups = 24 slots >= 23 angles/core; extra slots
  gather zeros).  Partition p = g*16+u plus block lane f in [0,32) covers
  nc = f*16+u.
- Per yx-chunk (64 positions): GPSIMD gathers [128, 2048] per step, DVE
  accumulates over steps, PE reduces the 8 groups with an exact 0/1 fp32
  selection matmul into PSUM, ACT copies PSUM->SBUF, sync DMA dumps to HBM.
- Raw Bass blocks with explicit semaphores (this walrus build allows at most
  one attached sync-wait per engine instruction; standalone EVSEM waits are
  used instead).
"""

from contextlib import ExitStack

import numpy as np

import concourse.bass as bass
from concourse import mybir
from concourse.bass_utils import run_bass_kernel_spmd

# Problem constants (hardcoded per the harness contract).
N, C, A, R = 4, 128, 180, 184
H = W = 128
YX = H * W  # 16384
NCORES = 8

GROUPS = 8  # 16-partition groups
U = 16  # partitions per group
F = 32  # nc values packed per rho block (IC inner size)
NCPC = F * U  # 512 nc rows held per core (all of them)
STEPS = 3  # angle slots per core = STEPS*GROUPS = 24 >= ceil(180/8)
ASLOT = STEPS * GROUPS  # 24
RPAD = R + 1  # 185 blocks; block R is all-zeros (invalid sink)
CHUNK = 64  # yx positions per chunk
NCH = YX // CHUNK  # 256 chunks
DW = RPAD * F  # data words per step per partition (5920)
CW = CHUNK // U  # idx columns per (step, chunk) per partition (4)
ICSUB = 1024 // F  # indices per IC (32): ISA caps IC dst at 1024 elems
NSUB = CHUNK // ICSUB  # sub-ICs per (chunk, step) (2)

_cache = {}


def _core_angles(core):
    """Angle slots for this core: slot t (0..23) -> global k or None."""
    ks = [k for k in range(A) if k % NCORES == core]
    return [ks[t] if t < len(ks) else None for t in range(ASLOT)]


def _rho_block_table():
    """[A, YX] int32 block indices into the padded rho axis (R = invalid)."""
    if "blk" in _cache:
        return _cache["blk"]
    k = np.arange(A)
    theta = k * (np.pi / A)
    cos_t = np.cos(theta)
    sin_t = np.sin(theta)
    y, x = np.meshgrid(np.arange(H), np.arange(W), indexing="ij")
    xc = (x - W // 2).astype(np.float64)
    yc = (y - H // 2).astype(np.float64)
    r = np.round(cos_t[:, None, None] * xc[None] + sin_t[:, None, None] * yc[None])
    r = r.astype(np.int64) + R // 2  # [A, H, W]
    valid = (r >= 0) & (r < R)
    blk = np.where(valid, np.clip(r, 0, R - 1), R).astype(np.int32)
    _cache["blk"] = blk.reshape(A, YX)
    return _cache["blk"]


def _idx_table(core):
    """uint16 idx stream for one core, SBUF layout [128, STEPS*NCH*CW].

    IndirectCopy unwraps a 16-partition group's idx tile as
    (col*16 + p_in_group); values are flat element offsets = block*F.
    Column layout: col = s*(NCH*CW) + q*CW + w.
    """
    key = ("idx", core)
    if key in _cache:
        return _cache[key]
    blk = _rho_block_table()
    angles = _core_angles(core)
    out = np.full((128, STEPS * NCH * CW), R * F, np.uint16)
    for s in range(STEPS):
        for g in range(GROUPS):
            k = angles[s * GROUPS + g]
            if k is None:
                continue
            flat = (blk[k] * F).astype(np.uint16)  # [YX]
            v = flat.reshape(NCH, CW, U)  # [q, w, p_in_group]
            v = v.transpose(2, 0, 1).reshape(U, NCH * CW)
            cols = slice(s * NCH * CW, (s + 1) * NCH * CW)
            out[g * U : (g + 1) * U, cols] = v
    _cache[key] = out
    return out


def _data_for_core(acc_flat, core):
    """acc_flat [512, A, R] f32 -> [128, STEPS*DW] f32 padded gather table.

    data[g*16+u, s*DW + rb*F + f] = acc_flat[f*16+u, k(s*8+g), rb]
    """
    angles = _core_angles(core)
    ac = np.zeros((NCPC, ASLOT, RPAD), np.float32)
    for t, k in enumerate(angles):
        if k is not None:
            ac[:, t, :R] = acc_flat[:, k, :]
    t = ac.reshape(F, U, ASLOT, RPAD)  # [f, u, t, r]
    t = t.transpose(2, 1, 3, 0)  # [t, u, r, f]
    t = t.reshape(STEPS, GROUPS, U, RPAD, F)  # [s, g, u, r, f]
    t = t.reshape(STEPS, 128, DW)
    return np.ascontiguousarray(t.transpose(1, 0, 2).reshape(128, STEPS * DW))


def _sel_matrix():
    """[128, 16] f32 selection: S[p, m] = 1 if p % 16 == m (group reduce)."""
    s = np.zeros((128, U), np.float32)
    s[np.arange(128), np.arange(128) % U] = 1.0
    return s


def _build_nc():
    if "nc" in _cache:
        return _cache["nc"]
    nc = bass.Bass("TRN2", debug=False, target_bir_lowering=False, num_devices=NCORES)
    data_d = nc.dram_tensor(
        "data", [128, STEPS * DW], mybir.dt.float32, kind="ExternalInput"
    ).ap()
    idx_d = nc.dram_tensor(
        "idx", [128, STEPS * NCH * CW], mybir.dt.uint16, kind="ExternalInput"
    ).ap()
    sel_d = nc.dram_tensor(
        "sel", [128, U], mybir.dt.float32, kind="ExternalInput"
    ).ap()
    raw_d = nc.dram_tensor(
        "raw", [NCH, U, CHUNK * F], mybir.dt.float32, kind="ExternalOutput"
    ).ap()

    GW = CHUNK * F  # 2048 gather/accum words per partition per (chunk, step)
    NMM = GW // 512  # matmuls per chunk (PSUM bank = 512 fp32)

    ctx = ExitStack()
    _cache["ctx"] = ctx
    data_sb = ctx.enter_context(nc.sbuf_tensor("data_sb", [128, STEPS * DW], mybir.dt.float32))
    idx_sb = ctx.enter_context(
        nc.sbuf_tensor("idx_sb", [128, STEPS * NCH * CW], mybir.dt.uint16)
    )
    sel_sb = ctx.enter_context(nc.sbuf_tensor("sel_sb", [128, U], mybir.dt.float32))
    NBUF = 3
    gbuf = [
        ctx.enter_context(nc.sbuf_tensor(f"gbuf{i}", [128, GW], mybir.dt.float32))
        for i in range(NBUF)
    ]
    abuf = [
        ctx.enter_context(nc.sbuf_tensor(f"abuf{i}", [128, GW], mybir.dt.float32))
        for i in range(2)
    ]
    obuf = [
        ctx.enter_context(nc.sbuf_tensor(f"obuf{i}", [U, GW], mybir.dt.float32))
        for i in range(2)
    ]
    psum = [
        ctx.enter_context(nc.psum_tensor(f"ps{i}", [U, GW], mybir.dt.float32))
        for i in range(2)
    ]
    ld_sem = ctx.enter_context(nc.semaphore("ld_sem"))
    ic_sem = ctx.enter_context(nc.semaphore("ic_sem"))
    add_sem = ctx.enter_context(nc.semaphore("add_sem"))
    mm_sem = ctx.enter_context(nc.semaphore("mm_sem"))
    cp_sem = ctx.enter_context(nc.semaphore("cp_sem"))
    dump_sem = ctx.enter_context(nc.semaphore("dump_sem"))
    block = ctx.enter_context(nc.Block())

    @block.gpsimd
    def _(gpsimd):
        gpsimd.dma_start(data_sb[:], data_d[:]).then_inc(ld_sem, 16)
        gpsimd.dma_start(idx_sb[:], idx_d[:]).then_inc(ld_sem, 16)
        gpsimd.dma_start(sel_sb[:], sel_d[:]).then_inc(ld_sem, 16)
        gpsimd.wait_ge(ld_sem, 48)
        jg = 0  # gather-tile slot counter (s>0 only)
        for q in range(NCH):
            for s in range(STEPS):
                if s == 0:
                    # s=0 gathers write the accumulator directly; PE must be
                    # done reading chunk q-2's accum slot first.
                    if q >= 2:
                        gpsimd.wait_ge(mm_sem, (q - 1) * NMM)
                    dst = abuf[q % 2]
                else:
                    if jg >= NBUF:
                        gpsimd.wait_ge(add_sem, jg - NBUF + 1)
                    dst = gbuf[jg % NBUF]
                    jg += 1
                dslice = data_sb[:, s * DW : (s + 1) * DW]
                ibase = s * NCH * CW + q * CW
                for sub in range(NSUB):
                    cw2 = CW // NSUB  # idx cols per sub-IC
                    isl = idx_sb[
                        :, ibase + sub * cw2 : ibase + (sub + 1) * cw2
                    ]
                    gpsimd.indirect_copy(
                        out=dst[
                            :, sub * (GW // NSUB) : (sub + 1) * (GW // NSUB)
                        ].rearrange("p (i f) -> p i f", f=F),
                        data=dslice.rearrange("p (r f) -> p r f", f=F),
                        idxs=isl,
                        i_know_ap_gather_is_preferred=True,
                    ).then_inc(ic_sem, 1)

    @block.vector
    def _(vector):
        jg = 0
        for q in range(NCH):
            acc = abuf[q % 2]
            for s in range(1, STEPS):
                # all sub-ICs through (q, s) done (covers the s=0 writes too)
                vector.wait_ge(ic_sem, (q * STEPS + s + 1) * NSUB)
                vector.tensor_add(acc[:], acc[:], gbuf[jg % NBUF][:]).then_inc(
                    add_sem, 1
                )
                jg += 1

    @block.tensor
    def _(tensor):
        for q in range(NCH):
            tensor.wait_ge(add_sem, (q + 1) * (STEPS - 1))
            if q >= 2:
                tensor.wait_ge(cp_sem, q - 1)  # psum slot reused
            for m in range(NMM):
                tensor.matmul(
                    out=psum[q % 2][:, m * 512 : (m + 1) * 512],
                    lhsT=sel_sb[:],
                    rhs=abuf[q % 2][:, m * 512 : (m + 1) * 512],
                    start=True,
                    stop=True,
                ).then_inc(mm_sem, 1)

    @block.scalar
    def _(scalar):
        for q in range(NCH):
            scalar.wait_ge(mm_sem, (q + 1) * NMM)
            if q >= 2:
                scalar.wait_ge(dump_sem, (q - 1) * 16)  # obuf slot reused
            scalar.copy(obuf[q % 2][:], psum[q % 2][:]).then_inc(cp_sem, 1)

    @block.sync
    def _(sync):
        for q in range(NCH):
            sync.wait_ge(cp_sem, q + 1)
            sync.dma_start(raw_d[q], obuf[q % 2][:]).then_inc(dump_sem, 16)

    _cache["nc"] = nc
    return nc


def _install_ntff_hook():
    """Provide the antenv.axon_hooks shim the image lacks, wiring the
    ctypes NTFF profiler from trn_agent_boot."""
    import sys
    import types

    if "antenv.axon_hooks" in sys.modules:
        return
    import antenv
    from trn_agent_boot.trn_boot import _ntff_profile_via_ctypes

    mod = types.ModuleType("antenv.axon_hooks")
    hook = _ntff_profile_via_ctypes("/opt/axon/libaxon_pjrt.so")
    mod.get_axon_ntff_profile_hook = lambda: hook
    mod.set_axon_ntff_profile_hook = lambda h: None
    sys.modules["antenv.axon_hooks"] = mod
    antenv.axon_hooks = mod


def hw_exec_time_ns(trace_cores=None):
    """Re-run the last kernel() invocation with tracing; return max core ns."""
    _install_ntff_hook()
    nc = _cache["nc"]
    res = run_bass_kernel_spmd(
        nc,
        _cache["in_maps"],
        core_ids=list(range(NCORES)),
        trace=True,
        trace_cores=trace_cores,
    )
    _cache["trace"] = res
    return res.exec_time_ns


def kernel(accumulator, out_H=128, out_W=128, numangle=180, numrho=184):
    accumulator = np.asarray(accumulator, np.float32)
    assert accumulator.shape == (N, C, A, R), accumulator.shape
    assert int(out_H) == H and int(out_W) == W
    assert int(numangle) == A and int(numrho) == R

    nc = _build_nc()
    acc_flat = np.ascontiguousarray(accumulator.reshape(N * C, A, R))
    sel = _sel_matrix()
    in_maps = [
        {
            "data": _data_for_core(acc_flat, core),
            "idx": _idx_table(core),
            "sel": sel,
        }
        for core in range(NCORES)
    ]
    _cache["in_maps"] = in_maps
    res = run_bass_kernel_spmd(nc, in_maps, core_ids=list(range(NCORES)))

    # Unshard: sum the 8 per-core partials.
    # raw[q, u, i*F + f] = partial for nc = f*16+u, yx = q*CHUNK+i
    total = np.zeros((NCPC, YX), np.float64)
    for core in range(NCORES):
        raw = res.results[core]["raw"]  # [NCH, U, CHUNK*F]
        oc = raw.reshape(NCH, U, CHUNK, F).transpose(3, 1, 0, 2).reshape(NCPC, YX)
        total += oc
    return total.astype(np.float32).reshape(N, C, H, W)


# revision 22
# speedup vs baseline: 1.0390x; 1.0039x over previous
"""Trainium2 Bass kernel for the inverse deep-hough-transform gather-reduce.

out[n, c, y, x] = sum_k acc[n, c, k, rho_idx[k, y, x]]  (masked by validity)

Design (v2)
-----------
- The rho index table is a pure function of static shapes; precomputed on the
  host.  Validity is folded in by pointing invalid entries at a zeroed spare
  slot (block R) of the data table.
- Gather primitive: GPSIMD IndirectCopy.  Measured cost is per 3-index
  read-request (~102-cycle serial RD_CMD), so each index fetches a contiguous
  block of F=32 nc-values -> 3.6us per 32-index IC (1024 elems/partition, the
  ISA cap per IC).
- Sharding: every core holds ALL 512 (n,c) rows; angles are sharded as
  k === core (mod 8).  Host sums the 8 per-core partials (the unshard step).
- Layout: 128 partitions = 8 groups x 16.  At step s, group g processes angle
  slot s*8+g (3 steps x 8 groups = 24 slots >= 23 angles/core; extra slots
  gather zeros).  Partition p = g*16+u plus block lane f in [0,32) covers
  nc = f*16+u.
- Per yx-chunk (64 positions): GPSIMD gathers [128, 2048] per step, DVE
  accumulates over steps, PE reduces the 8 groups with an exact 0/1 fp32
  selection matmul into PSUM, ACT copies PSUM->SBUF, sync DMA dumps to HBM.
- Raw Bass blocks with explicit semaphores (this walrus build allows at most
  one attached sync-wait per engine instruction; standalone EVSEM waits are
  used instead).
"""

from contextlib import ExitStack

import numpy as np

import concourse.bass as bass
from concourse import mybir
from concourse.bass_utils import run_bass_kernel_spmd

# Problem constants (hardcoded per the harness contract).
N, C, A, R = 4, 128, 180, 184
H = W = 128
YX = H * W  # 16384
NCORES = 8

GROUPS = 8  # 16-partition groups
U = 16  # partitions per group
F = 32  # nc values packed per rho block (IC inner size)
NCPC = F * U  # 512 nc rows held per core (all of them)
STEPS = 3  # angle slots per core = STEPS*GROUPS = 24 >= ceil(180/8)
ASLOT = STEPS * GROUPS  # 24
RPAD = R + 1  # 185 blocks; block R is all-zeros (invalid sink)
CHUNK = 64  # yx positions per chunk
NCH = YX // CHUNK  # 256 chunks
DW = RPAD * F  # data words per step per partition (5920)
CW = CHUNK // U  # idx columns per (step, chunk) per partition (4)
ICSUB = 1024 // F  # indices per IC (32): ISA caps IC dst at 1024 elems
NSUB = CHUNK // ICSUB  # sub-ICs per (chunk, step) (2)

_cache = {}


def _core_angles(core):
    """Angle slots for this core: slot t (0..23) -> global k or None."""
    ks = [k for k in range(A) if k % NCORES == core]
    return [ks[t] if t < len(ks) else None for t in range(ASLOT)]


def _rho_block_table():
    """[A, YX] int32 block indices into the padded rho axis (R = invalid)."""
    if "blk" in _cache:
        return _cache["blk"]
    k = np.arange(A)
    theta = k * (np.pi / A)
    cos_t = np.cos(theta)
    sin_t = np.sin(theta)
    y, x = np.meshgrid(np.arange(H), np.arange(W), indexing="ij")
    xc = (x - W // 2).astype(np.float64)
    yc = (y - H // 2).astype(np.float64)
    r = np.round(cos_t[:, None, None] * xc[None] + sin_t[:, None, None] * yc[None])
    r = r.astype(np.int64) + R // 2  # [A, H, W]
    valid = (r >= 0) & (r < R)
    blk = np.where(valid, np.clip(r, 0, R - 1), R).astype(np.int32)
    _cache["blk"] = blk.reshape(A, YX)
    return _cache["blk"]


def _idx_table(core):
    """uint16 idx stream for one core, SBUF layout [128, STEPS*NCH*CW].

    IndirectCopy unwraps a 16-partition group's idx tile as
    (col*16 + p_in_group); values are flat element offsets = block*F.
    Column layout: col = s*(NCH*CW) + q*CW + w.
    """
    key = ("idx", core)
    if key in _cache:
        return _cache[key]
    blk = _rho_block_table()
    angles = _core_angles(core)
    out = np.full((128, STEPS * NCH * CW), R * F, np.uint16)
    for s in range(STEPS):
        for g in range(GROUPS):
            k = angles[s * GROUPS + g]
            if k is None:
                continue
            flat = (blk[k] * F).astype(np.uint16)  # [YX]
            v = flat.reshape(NCH, CW, U)  # [q, w, p_in_group]
            v = v.transpose(2, 0, 1).reshape(U, NCH * CW)
            cols = slice(s * NCH * CW, (s + 1) * NCH * CW)
            out[g * U : (g + 1) * U, cols] = v
    _cache[key] = out
    return out


def _data_for_core(acc_flat, core):
    """acc_flat [512, A, R] f32 -> [128, STEPS*DW] f32 padded gather table.

    data[g*16+u, s*DW + rb*F + f] = acc_flat[f*16+u, k(s*8+g), rb]
    """
    angles = _core_angles(core)
    ac = np.zeros((NCPC, ASLOT, RPAD), np.float32)
    for t, k in enumerate(angles):
        if k is not None:
            ac[:, t, :R] = acc_flat[:, k, :]
    t = ac.reshape(F, U, ASLOT, RPAD)  # [f, u, t, r]
    t = t.transpose(2, 1, 3, 0)  # [t, u, r, f]
    t = t.reshape(STEPS, GROUPS, U, RPAD, F)  # [s, g, u, r, f]
    t = t.reshape(STEPS, 128, DW)
    return np.ascontiguousarray(t.transpose(1, 0, 2).reshape(128, STEPS * DW))


def _sel_matrix():
    """[128, 16] f32 selection: S[p, m] = 1 if p % 16 == m (group reduce)."""
    s = np.zeros((128, U), np.float32)
    s[np.arange(128), np.arange(128) % U] = 1.0
    return s


def _build_nc():
    if "nc" in _cache:
        return _cache["nc"]
    nc = bass.Bass("TRN2", debug=False, target_bir_lowering=False, num_devices=NCORES)
    data_d = nc.dram_tensor(
        "data", [128, STEPS * DW], mybir.dt.float32, kind="ExternalInput"
    ).ap()
    idx_d = nc.dram_tensor(
        "idx", [128, STEPS * NCH * CW], mybir.dt.uint16, kind="ExternalInput"
    ).ap()
    sel_d = nc.dram_tensor(
        "sel", [128, U], mybir.dt.float32, kind="ExternalInput"
    ).ap()
    raw_d = nc.dram_tensor(
        "raw", [NCH, U, CHUNK * F], mybir.dt.float32, kind="ExternalOutput"
    ).ap()

    GW = CHUNK * F  # 2048 gather/accum words per partition per (chunk, step)
    NMM = GW // 512  # matmuls per chunk (PSUM bank = 512 fp32)

    ctx = ExitStack()
    _cache["ctx"] = ctx
    data_sb = ctx.enter_context(nc.sbuf_tensor("data_sb", [128, STEPS * DW], mybir.dt.float32))
    idx_sb = ctx.enter_context(
        nc.sbuf_tensor("idx_sb", [128, STEPS * NCH * CW], mybir.dt.uint16)
    )
    sel_sb = ctx.enter_context(nc.sbuf_tensor("sel_sb", [128, U], mybir.dt.float32))
    NBUF = 3
    gbuf = [
        ctx.enter_context(nc.sbuf_tensor(f"gbuf{i}", [128, GW], mybir.dt.float32))
        for i in range(NBUF)
    ]
    abuf = [
        ctx.enter_context(nc.sbuf_tensor(f"abuf{i}", [128, GW], mybir.dt.float32))
        for i in range(3)
    ]
    obuf = [
        ctx.enter_context(nc.sbuf_tensor(f"obuf{i}", [U, GW], mybir.dt.float32))
        for i in range(2)
    ]
    psum = [
        ctx.enter_context(nc.psum_tensor(f"ps{i}", [U, GW], mybir.dt.float32))
        for i in range(2)
    ]
    ld_sem = ctx.enter_context(nc.semaphore("ld_sem"))
    ic_sem = ctx.enter_context(nc.semaphore("ic_sem"))
    add_sem = ctx.enter_context(nc.semaphore("add_sem"))
    mm_sem = ctx.enter_context(nc.semaphore("mm_sem"))
    cp_sem = ctx.enter_context(nc.semaphore("cp_sem"))
    dump_sem = ctx.enter_context(nc.semaphore("dump_sem"))
    block = ctx.enter_context(nc.Block())

    @block.gpsimd
    def _(gpsimd):
        gpsimd.dma_start(data_sb[:], data_d[:]).then_inc(ld_sem, 16)
        gpsimd.dma_start(idx_sb[:], idx_d[:]).then_inc(ld_sem, 16)
        gpsimd.dma_start(sel_sb[:], sel_d[:]).then_inc(ld_sem, 16)
        gpsimd.wait_ge(ld_sem, 48)
        jg = 0  # gather-tile slot counter (s>0 only)
        for q in range(NCH):
            for s in range(STEPS):
                if s == 0:
                    # s=0 gathers write the accumulator directly; PE must be
                    # done reading chunk q-3's accum slot first.
                    if q >= 3:
                        gpsimd.wait_ge(mm_sem, (q - 2) * NMM)
                    dst = abuf[q % 3]
                else:
                    if jg >= NBUF:
                        gpsimd.wait_ge(add_sem, jg - NBUF + 1)
                    dst = gbuf[jg % NBUF]
                    jg += 1
                dslice = data_sb[:, s * DW : (s + 1) * DW]
                ibase = s * NCH * CW + q * CW
                for sub in range(NSUB):
                    cw2 = CW // NSUB  # idx cols per sub-IC
                    isl = idx_sb[
                        :, ibase + sub * cw2 : ibase + (sub + 1) * cw2
                    ]
                    gpsimd.indirect_copy(
                        out=dst[
                            :, sub * (GW // NSUB) : (sub + 1) * (GW // NSUB)
                        ].rearrange("p (i f) -> p i f", f=F),
                        data=dslice.rearrange("p (r f) -> p r f", f=F),
                        idxs=isl,
                        i_know_ap_gather_is_preferred=True,
                    ).then_inc(ic_sem, 1)

    @block.vector
    def _(vector):
        jg = 0
        for q in range(NCH):
            acc = abuf[q % 3]
            for s in range(1, STEPS):
                # all sub-ICs through (q, s) done (covers the s=0 writes too)
                vector.wait_ge(ic_sem, (q * STEPS + s + 1) * NSUB)
                vector.tensor_add(acc[:], acc[:], gbuf[jg % NBUF][:]).then_inc(
                    add_sem, 1
                )
                jg += 1

    @block.tensor
    def _(tensor):
        for q in range(NCH):
            tensor.wait_ge(add_sem, (q + 1) * (STEPS - 1))
            if q >= 2:
                tensor.wait_ge(cp_sem, q - 1)  # psum slot reused
            for m in range(NMM):
                tensor.matmul(
                    out=psum[q % 2][:, m * 512 : (m + 1) * 512],
                    lhsT=sel_sb[:],
                    rhs=abuf[q % 3][:, m * 512 : (m + 1) * 512],
                    start=True,
                    stop=True,
                ).then_inc(mm_sem, 1)

    @block.scalar
    def _(scalar):
        for q in range(NCH):
            scalar.wait_ge(mm_sem, (q + 1) * NMM)
            if q >= 2:
                scalar.wait_ge(dump_sem, (q - 1) * 16)  # obuf slot reused
            scalar.copy(obuf[q % 2][:], psum[q % 2][:]).then_inc(cp_sem, 1)

    @block.sync
    def _(sync):
        for q in range(NCH):
            sync.wait_ge(cp_sem, q + 1)
            sync.dma_start(raw_d[q], obuf[q % 2][:]).then_inc(dump_sem, 16)

    _cache["nc"] = nc
    return nc


def _install_ntff_hook():
    """Provide the antenv.axon_hooks shim the image lacks, wiring the
    ctypes NTFF profiler from trn_agent_boot."""
    import sys
    import types

    if "antenv.axon_hooks" in sys.modules:
        return
    import antenv
    from trn_agent_boot.trn_boot import _ntff_profile_via_ctypes

    mod = types.ModuleType("antenv.axon_hooks")
    hook = _ntff_profile_via_ctypes("/opt/axon/libaxon_pjrt.so")
    mod.get_axon_ntff_profile_hook = lambda: hook
    mod.set_axon_ntff_profile_hook = lambda h: None
    sys.modules["antenv.axon_hooks"] = mod
    antenv.axon_hooks = mod


def hw_exec_time_ns(trace_cores=None):
    """Re-run the last kernel() invocation with tracing; return max core ns."""
    _install_ntff_hook()
    nc = _cache["nc"]
    res = run_bass_kernel_spmd(
        nc,
        _cache["in_maps"],
        core_ids=list(range(NCORES)),
        trace=True,
        trace_cores=trace_cores,
    )
    _cache["trace"] = res
    return res.exec_time_ns


def kernel(accumulator, out_H=128, out_W=128, numangle=180, numrho=184):
    accumulator = np.asarray(accumulator, np.float32)
    assert accumulator.shape == (N, C, A, R), accumulator.shape
    assert int(out_H) == H and int(out_W) == W
    assert int(numangle) == A and int(numrho) == R

    nc = _build_nc()
    acc_flat = np.ascontiguousarray(accumulator.reshape(N * C, A, R))
    sel = _sel_matrix()
    in_maps = [
        {
            "data": _data_for_core(acc_flat, core),
            "idx": _idx_table(core),
            "sel": sel,
        }
        for core in range(NCORES)
    ]
    _cache["in_maps"] = in_maps
    res = run_bass_kernel_spmd(nc, in_maps, core_ids=list(range(NCORES)))

    # Unshard: sum the 8 per-core partials.
    # raw[q, u, i*F + f] = partial for nc = f*16+u, yx = q*CHUNK+i
    total = np.zeros((NCPC, YX), np.float64)
    for core in range(NCORES):
        raw = res.results[core]["raw"]  # [NCH, U, CHUNK*F]
        oc = raw.reshape(NCH, U, CHUNK, F).transpose(3, 1, 0, 2).reshape(NCPC, YX)
        total += oc
    return total.astype(np.float32).reshape(N, C, H, W)


# revision 23
# speedup vs baseline: 1.0394x; 1.0003x over previous
"""Trainium2 Bass kernel for the inverse deep-hough-transform gather-reduce.

out[n, c, y, x] = sum_k acc[n, c, k, rho_idx[k, y, x]]  (masked by validity)

Design (v2)
-----------
- The rho index table is a pure function of static shapes; precomputed on the
  host.  Validity is folded in by pointing invalid entries at a zeroed spare
  slot (block R) of the data table.
- Gather primitive: GPSIMD IndirectCopy.  Measured cost is per 3-index
  read-request (~102-cycle serial RD_CMD), so each index fetches a contiguous
  block of F=32 nc-values -> 3.6us per 32-index IC (1024 elems/partition, the
  ISA cap per IC).
- Sharding: every core holds ALL 512 (n,c) rows; angles are sharded as
  k === core (mod 8).  Host sums the 8 per-core partials (the unshard step).
- Layout: 128 partitions = 8 groups x 16.  At step s, group g processes angle
  slot s*8+g (3 steps x 8 groups = 24 slots >= 23 angles/core; extra slots
  gather zeros).  Partition p = g*16+u plus block lane f in [0,32) covers
  nc = f*16+u.
- Per yx-chunk (64 positions): GPSIMD gathers [128, 2048] per step, DVE
  accumulates over steps, PE reduces the 8 groups with an exact 0/1 fp32
  selection matmul into PSUM, ACT copies PSUM->SBUF, sync DMA dumps to HBM.
- Raw Bass blocks with explicit semaphores (this walrus build allows at most
  one attached sync-wait per engine instruction; standalone EVSEM waits are
  used instead).
"""

from contextlib import ExitStack

import numpy as np

import concourse.bass as bass
from concourse import mybir
from concourse.bass_utils import run_bass_kernel_spmd

# Problem constants (hardcoded per the harness contract).
N, C, A, R = 4, 128, 180, 184
H = W = 128
YX = H * W  # 16384
NCORES = 8

GROUPS = 8  # 16-partition groups
U = 16  # partitions per group
F = 32  # nc values packed per rho block (IC inner size)
NCPC = F * U  # 512 nc rows held per core (all of them)
STEPS = 3  # angle slots per core = STEPS*GROUPS = 24 >= ceil(180/8)
ASLOT = STEPS * GROUPS  # 24
RPAD = R + 1  # 185 blocks; block R is all-zeros (invalid sink)
CHUNK = 64  # yx positions per chunk
NCH = YX // CHUNK  # 256 chunks
DW = RPAD * F  # data words per step per partition (5920)
CW = CHUNK // U  # idx columns per (step, chunk) per partition (4)
ICSUB = 1024 // F  # indices per IC (32): ISA caps IC dst at 1024 elems
NSUB = CHUNK // ICSUB  # sub-ICs per (chunk, step) (2)

_cache = {}


def _core_angles(core):
    """Angle slots for this core: slot t (0..23) -> global k or None."""
    ks = [k for k in range(A) if k % NCORES == core]
    return [ks[t] if t < len(ks) else None for t in range(ASLOT)]


def _rho_block_table():
    """[A, YX] int32 block indices into the padded rho axis (R = invalid)."""
    if "blk" in _cache:
        return _cache["blk"]
    k = np.arange(A)
    theta = k * (np.pi / A)
    cos_t = np.cos(theta)
    sin_t = np.sin(theta)
    y, x = np.meshgrid(np.arange(H), np.arange(W), indexing="ij")
    xc = (x - W // 2).astype(np.float64)
    yc = (y - H // 2).astype(np.float64)
    r = np.round(cos_t[:, None, None] * xc[None] + sin_t[:, None, None] * yc[None])
    r = r.astype(np.int64) + R // 2  # [A, H, W]
    valid = (r >= 0) & (r < R)
    blk = np.where(valid, np.clip(r, 0, R - 1), R).astype(np.int32)
    _cache["blk"] = blk.reshape(A, YX)
    return _cache["blk"]


def _idx_table(core):
    """uint16 idx stream for one core, SBUF layout [128, STEPS*NCH*CW].

    IndirectCopy unwraps a 16-partition group's idx tile as
    (col*16 + p_in_group); values are flat element offsets = block*F.
    Column layout: col = s*(NCH*CW) + q*CW + w.
    """
    key = ("idx", core)
    if key in _cache:
        return _cache[key]
    blk = _rho_block_table()
    angles = _core_angles(core)
    out = np.full((128, STEPS * NCH * CW), R * F, np.uint16)
    for s in range(STEPS):
        for g in range(GROUPS):
            k = angles[s * GROUPS + g]
            if k is None:
                continue
            flat = (blk[k] * F).astype(np.uint16)  # [YX]
            v = flat.reshape(NCH, CW, U)  # [q, w, p_in_group]
            v = v.transpose(2, 0, 1).reshape(U, NCH * CW)
            cols = slice(s * NCH * CW, (s + 1) * NCH * CW)
            out[g * U : (g + 1) * U, cols] = v
    _cache[key] = out
    return out


def _data_for_core(acc_flat, core):
    """acc_flat [512, A, R] f32 -> [128, STEPS*DW] f32 padded gather table.

    data[g*16+u, s*DW + rb*F + f] = acc_flat[f*16+u, k(s*8+g), rb]
    """
    angles = _core_angles(core)
    ac = np.zeros((NCPC, ASLOT, RPAD), np.float32)
    for t, k in enumerate(angles):
        if k is not None:
            ac[:, t, :R] = acc_flat[:, k, :]
    t = ac.reshape(F, U, ASLOT, RPAD)  # [f, u, t, r]
    t = t.transpose(2, 1, 3, 0)  # [t, u, r, f]
    t = t.reshape(STEPS, GROUPS, U, RPAD, F)  # [s, g, u, r, f]
    t = t.reshape(STEPS, 128, DW)
    return np.ascontiguousarray(t.transpose(1, 0, 2).reshape(128, STEPS * DW))


def _sel_matrix():
    """[128, 16] f32 selection: S[p, m] = 1 if p % 16 == m (group reduce)."""
    s = np.zeros((128, U), np.float32)
    s[np.arange(128), np.arange(128) % U] = 1.0
    return s


def _build_nc():
    if "nc" in _cache:
        return _cache["nc"]
    nc = bass.Bass("TRN2", debug=False, target_bir_lowering=False, num_devices=NCORES)
    data_d = nc.dram_tensor(
        "data", [128, STEPS * DW], mybir.dt.float32, kind="ExternalInput"
    ).ap()
    idx_d = nc.dram_tensor(
        "idx", [128, STEPS * NCH * CW], mybir.dt.uint16, kind="ExternalInput"
    ).ap()
    sel_d = nc.dram_tensor(
        "sel", [128, U], mybir.dt.float32, kind="ExternalInput"
    ).ap()
    raw_d = nc.dram_tensor(
        "raw", [NCH, U, CHUNK * F], mybir.dt.float32, kind="ExternalOutput"
    ).ap()

    GW = CHUNK * F  # 2048 gather/accum words per partition per (chunk, step)
    NMM = GW // 512  # matmuls per chunk (PSUM bank = 512 fp32)

    ctx = ExitStack()
    _cache["ctx"] = ctx
    data_sb = ctx.enter_context(nc.sbuf_tensor("data_sb", [128, STEPS * DW], mybir.dt.float32))
    idx_sb = ctx.enter_context(
        nc.sbuf_tensor("idx_sb", [128, STEPS * NCH * CW], mybir.dt.uint16)
    )
    sel_sb = ctx.enter_context(nc.sbuf_tensor("sel_sb", [128, U], mybir.dt.float32))
    NBUF = 3
    gbuf = [
        ctx.enter_context(nc.sbuf_tensor(f"gbuf{i}", [128, GW], mybir.dt.float32))
        for i in range(NBUF)
    ]
    abuf = [
        ctx.enter_context(nc.sbuf_tensor(f"abuf{i}", [128, GW], mybir.dt.float32))
        for i in range(3)
    ]
    obuf = [
        ctx.enter_context(nc.sbuf_tensor(f"obuf{i}", [U, GW], mybir.dt.float32))
        for i in range(2)
    ]
    psum = [
        ctx.enter_context(nc.psum_tensor(f"ps{i}", [U, GW], mybir.dt.float32))
        for i in range(2)
    ]
    ld_sem = ctx.enter_context(nc.semaphore("ld_sem"))
    ic_sem = ctx.enter_context(nc.semaphore("ic_sem"))
    add_sem = ctx.enter_context(nc.semaphore("add_sem"))
    mm_sem = ctx.enter_context(nc.semaphore("mm_sem"))
    cp_sem = ctx.enter_context(nc.semaphore("cp_sem"))
    dump_sem = ctx.enter_context(nc.semaphore("dump_sem"))
    block = ctx.enter_context(nc.Block())

    @block.gpsimd
    def _(gpsimd):
        gpsimd.dma_start(data_sb[:], data_d[:]).then_inc(ld_sem, 16)
        gpsimd.dma_start(idx_sb[:], idx_d[:]).then_inc(ld_sem, 16)
        gpsimd.dma_start(sel_sb[:], sel_d[:]).then_inc(ld_sem, 16)
        gpsimd.wait_ge(ld_sem, 48)
        jg = 0  # gather-tile slot counter (s>0 only)
        for q in range(NCH):
            # Single pool wait per chunk: add (q-1, s=1) done.  This frees
            # both gbuf slots used this chunk, and (because that add itself
            # waits mm_sem >= (q-3)*NMM on the DVE stream) transitively
            # guarantees PE is done reading abuf[q % 3].
            if q >= 2:
                gpsimd.wait_ge(add_sem, 2 * q - 1)
            for s in range(STEPS):
                if s == 0:
                    dst = abuf[q % 3]
                else:
                    dst = gbuf[jg % NBUF]
                    jg += 1
                dslice = data_sb[:, s * DW : (s + 1) * DW]
                ibase = s * NCH * CW + q * CW
                for sub in range(NSUB):
                    cw2 = CW // NSUB  # idx cols per sub-IC
                    isl = idx_sb[
                        :, ibase + sub * cw2 : ibase + (sub + 1) * cw2
                    ]
                    gpsimd.indirect_copy(
                        out=dst[
                            :, sub * (GW // NSUB) : (sub + 1) * (GW // NSUB)
                        ].rearrange("p (i f) -> p i f", f=F),
                        data=dslice.rearrange("p (r f) -> p r f", f=F),
                        idxs=isl,
                        i_know_ap_gather_is_preferred=True,
                    ).then_inc(ic_sem, 1)

    @block.vector
    def _(vector):
        jg = 0
        for q in range(NCH):
            acc = abuf[q % 3]
            for s in range(1, STEPS):
                # all sub-ICs through (q, s) done (covers the s=0 writes too)
                vector.wait_ge(ic_sem, (q * STEPS + s + 1) * NSUB)
                if s == 1 and q >= 3:
                    # Carrier wait: lets the pool infer PE-done(q-3) from
                    # add_sem alone (see gpsimd stream).
                    vector.wait_ge(mm_sem, (q - 2) * NMM)
                vector.tensor_add(acc[:], acc[:], gbuf[jg % NBUF][:]).then_inc(
                    add_sem, 1
                )
                jg += 1

    @block.tensor
    def _(tensor):
        for q in range(NCH):
            tensor.wait_ge(add_sem, (q + 1) * (STEPS - 1))
            if q >= 2:
                tensor.wait_ge(cp_sem, q - 1)  # psum slot reused
            for m in range(NMM):
                tensor.matmul(
                    out=psum[q % 2][:, m * 512 : (m + 1) * 512],
                    lhsT=sel_sb[:],
                    rhs=abuf[q % 3][:, m * 512 : (m + 1) * 512],
                    start=True,
                    stop=True,
                ).then_inc(mm_sem, 1)

    @block.scalar
    def _(scalar):
        for q in range(NCH):
            scalar.wait_ge(mm_sem, (q + 1) * NMM)
            if q >= 2:
                scalar.wait_ge(dump_sem, (q - 1) * 16)  # obuf slot reused
            scalar.copy(obuf[q % 2][:], psum[q % 2][:]).then_inc(cp_sem, 1)

    @block.sync
    def _(sync):
        for q in range(NCH):
            sync.wait_ge(cp_sem, q + 1)
            sync.dma_start(raw_d[q], obuf[q % 2][:]).then_inc(dump_sem, 16)

    _cache["nc"] = nc
    return nc


def _install_ntff_hook():
    """Provide the antenv.axon_hooks shim the image lacks, wiring the
    ctypes NTFF profiler from trn_agent_boot."""
    import sys
    import types

    if "antenv.axon_hooks" in sys.modules:
        return
    import antenv
    from trn_agent_boot.trn_boot import _ntff_profile_via_ctypes

    mod = types.ModuleType("antenv.axon_hooks")
    hook = _ntff_profile_via_ctypes("/opt/axon/libaxon_pjrt.so")
    mod.get_axon_ntff_profile_hook = lambda: hook
    mod.set_axon_ntff_profile_hook = lambda h: None
    sys.modules["antenv.axon_hooks"] = mod
    antenv.axon_hooks = mod


def hw_exec_time_ns(trace_cores=None):
    """Re-run the last kernel() invocation with tracing; return max core ns."""
    _install_ntff_hook()
    nc = _cache["nc"]
    res = run_bass_kernel_spmd(
        nc,
        _cache["in_maps"],
        core_ids=list(range(NCORES)),
        trace=True,
        trace_cores=trace_cores,
    )
    _cache["trace"] = res
    return res.exec_time_ns


def kernel(accumulator, out_H=128, out_W=128, numangle=180, numrho=184):
    accumulator = np.asarray(accumulator, np.float32)
    assert accumulator.shape == (N, C, A, R), accumulator.shape
    assert int(out_H) == H and int(out_W) == W
    assert int(numangle) == A and int(numrho) == R

    nc = _build_nc()
    acc_flat = np.ascontiguousarray(accumulator.reshape(N * C, A, R))
    sel = _sel_matrix()
    in_maps = [
        {
            "data": _data_for_core(acc_flat, core),
            "idx": _idx_table(core),
            "sel": sel,
        }
        for core in range(NCORES)
    ]
    _cache["in_maps"] = in_maps
    res = run_bass_kernel_spmd(nc, in_maps, core_ids=list(range(NCORES)))

    # Unshard: sum the 8 per-core partials.
    # raw[q, u, i*F + f] = partial for nc = f*16+u, yx = q*CHUNK+i
    total = np.zeros((NCPC, YX), np.float64)
    for core in range(NCORES):
        raw = res.results[core]["raw"]  # [NCH, U, CHUNK*F]
        oc = raw.reshape(NCH, U, CHUNK, F).transpose(3, 1, 0, 2).reshape(NCPC, YX)
        total += oc
    return total.astype(np.float32).reshape(N, C, H, W)
